# revision 16
# baseline (speedup 1.0000x reference)
"""Additive (Bahdanau) attention on 8 TRN2 NeuronCores.

reference:
    q = query @ Wq.T + bq                  [B, Lq, H]
    k = key @ Wk.T + bk                    [B, Lk, H]
    scores[b,q,k] = sum_h v[h] * tanh(qp[b,q,h] + kp[b,k,h]) (+ bv)
    scores = where(mask==0, -inf, scores)
    attn_w = softmax(scores, axis=-1)      [B, Lq, Lk]
    attn_out = attn_w @ value              [B, Lq, H]
    returns (attn_out, attn_w)

Sharding: B*Lq = 1024 query rows split 8 ways -> 128 rows/core, each core
gets its batch's key/value/mask. Zero cross-core communication.

Per-core dataflow (h on partitions for the tanh pipeline):
  - PE transposes query/key/Wq/Wk tiles; PE computes qpT[h,q], kpT[h,k].
  - main loop over (h-tile, q-block): DVE tensor_scalar_add broadcasts
    qpT[:,q] over kpT -> big S_in tile; ScalarE tanh (the hard floor:
    Lq*Lk*H/8 = 16.8M elements through the only transcendental engine);
    PE reduces over h with v as a [128,1] stationary operand into PSUM
    score rows.
  - masked softmax on the [128 q, 512 k] PSUM tile (exp shares the tanh
    ACT table set; accum_out yields row sums in the same instruction).
  - PE transposes attn_w, matmuls against value, DMA out.

Note bv is mathematically irrelevant: softmax is shift invariant and
scores are not returned.
"""

from contextlib import ExitStack

import numpy as np

import concourse.bass as bass
import concourse.tile as tile
from concourse import bacc, mybir
from concourse.masks import make_identity

B, LQ, LK, H = 4, 256, 512, 256
NCORES = 8
QROWS = B * LQ // NCORES  # 128 query rows per core
QB = 8                    # query rows per activation block
NEG_BIG = -1e30

F32 = mybir.dt.float32
I32 = mybir.dt.int32

_CACHE: dict = {}


def _build_nc():
    nc = bacc.Bacc()

    query = nc.declare_dram_parameter("query", [QROWS, H], F32, isOutput=False)
    key = nc.declare_dram_parameter("key", [LK, H], F32, isOutput=False)
    value = nc.declare_dram_parameter("value", [LK, H], F32, isOutput=False)
    mask = nc.declare_dram_parameter("mask", [LK], I32, isOutput=False)
    Wq = nc.declare_dram_parameter("Wq", [H, H], F32, isOutput=False)
    bq = nc.declare_dram_parameter("bq", [H], F32, isOutput=False)
    Wk = nc.declare_dram_parameter("Wk", [H, H], F32, isOutput=False)
    bk = nc.declare_dram_parameter("bk", [H], F32, isOutput=False)
    v = nc.declare_dram_parameter("v", [H], F32, isOutput=False)

    attn_out = nc.declare_dram_parameter("attn_out", [QROWS, H], F32, isOutput=True)
    attn_w = nc.declare_dram_parameter("attn_w", [QROWS, LK], F32, isOutput=True)

    HT = H // 128  # h tiles (2)
    IT = H // 128  # hin tiles (2)
    KT = LK // 128  # key row tiles (4)
    NBLK = QROWS // QB

    with tile.TileContext(nc) as tc, ExitStack() as ctx:
        persist = ctx.enter_context(tc.tile_pool(name="persist", bufs=1))
        sin_pool = ctx.enter_context(tc.tile_pool(name="sin", bufs=2))
        tb_pool = ctx.enter_context(tc.tile_pool(name="tb", bufs=2))
        tp_ps = ctx.enter_context(tc.tile_pool(name="tp_ps", bufs=2, space="PSUM"))
        pj_ps = ctx.enter_context(tc.tile_pool(name="pj_ps", bufs=2, space="PSUM"))
        sc_ps = ctx.enter_context(tc.tile_pool(name="sc_ps", bufs=1, space="PSUM"))
        out_ps = ctx.enter_context(tc.tile_pool(name="out_ps", bufs=1, space="PSUM"))

        # ---- constants / identity / ACT table warmup -------------------
        # NOTE: walrus allows only ONE semaphore wait per Matmult, so every
        # tile the PE reads is routed through a DVE copy: PE instructions
        # then only ever wait on the (cumulative) Vector semaphore and the
        # ACT semaphore, one fresh clock at a time.
        ident_g = persist.tile([128, 128], F32)
        make_identity(nc, ident_g)
        ident = persist.tile([128, 128], F32)
        nc.vector.tensor_copy(ident, ident_g)
        warm = persist.tile([128, 1], F32)
        nc.vector.memset(warm, 0.0)
        # touch the exp_and_others table set early (covers tanh + exp)
        nc.scalar.activation(warm, warm, mybir.ActivationFunctionType.Tanh)

        # ---- raw loads (DMA -> staging, DVE copy -> PE-visible) --------
        def load_pe(src_ap, shape, tag):
            d = persist.tile(shape, F32, tag=f"{tag}_d")
            nc.sync.dma_start(out=d, in_=src_ap)
            t = persist.tile(shape, F32, tag=tag)
            nc.vector.tensor_copy(t, d)
            return t

        wq_raw = [load_pe(Wq[hr * 128:(hr + 1) * 128, :], [128, H], f"wq_raw{hr}")
                  for hr in range(HT)]
        wk_raw = [load_pe(Wk[hr * 128:(hr + 1) * 128, :], [128, H], f"wk_raw{hr}")
                  for hr in range(HT)]
        query_sb = load_pe(query[:, :], [128, H], "query_sb")
        key_sb = [load_pe(key[kt * 128:(kt + 1) * 128, :], [128, H], f"key_sb{kt}")
                  for kt in range(KT)]
        value_sb = [load_pe(value[kt * 128:(kt + 1) * 128, :], [128, H], f"value_sb{kt}")
                    for kt in range(KT)]

        # mask broadcast to all partitions, then to additive bias
        mask_i = persist.tile([128, LK], I32)
        mask_bcast = bass.AP(tensor=mask, offset=0, ap=[[0, 128], [1, LK]])
        nc.sync.dma_start(out=mask_i, in_=mask_bcast)
        mbias = persist.tile([128, LK], F32)
        # mask==1 -> 0.0 ; mask==0 -> NEG_BIG
        nc.vector.tensor_scalar(
            out=mbias, in0=mask_i, scalar1=-NEG_BIG, scalar2=NEG_BIG,
            op0=mybir.AluOpType.mult, op1=mybir.AluOpType.add,
        )

        # per-partition columns of bq, bk, v (strided DMA: partition stride 1)
        def load_col(src, ht, tag):
            d = persist.tile([128, 1], F32, tag=f"{tag}_d")
            col = bass.AP(tensor=src, offset=ht * 128, ap=[[1, 128], [0, 1]])
            nc.sync.dma_start(out=d, in_=col)
            t = persist.tile([128, 1], F32, tag=tag)
            nc.vector.tensor_copy(t, d)
            return t

        bq_col = [load_col(bq, ht, f"bq{ht}") for ht in range(HT)]
        bk_col = [load_col(bk, ht, f"bk{ht}") for ht in range(HT)]

        # v columns copied via ScalarE: the scores matmuls then have both
        # operands ACT-produced -> a single cumulative ACT wait.
        def load_col_act(src, ht, tag):
            d = persist.tile([128, 1], F32, tag=f"{tag}_d")
            col = bass.AP(tensor=src, offset=ht * 128, ap=[[1, 128], [0, 1]])
            nc.sync.dma_start(out=d, in_=col)
            t = persist.tile([128, 1], F32, tag=tag)
            nc.scalar.copy(t, d)
            return t

        v_col = [load_col_act(v, ht, f"v{ht}") for ht in range(HT)]
        bsum = []
        for ht in range(HT):
            t = persist.tile([128, 1], F32, tag=f"bsum{ht}")
            nc.vector.tensor_add(t, bq_col[ht], bk_col[ht])
            bsum.append(t)

        # ---- transposes (PE) -------------------------------------------
        def transpose_tiles(src_tiles, n_colblocks, dst_cols, tag):
            """src_tiles: list of [128, n_colblocks*128] sbuf tiles (rows r).
            Returns list over colblock it of [128, len(src_tiles)*128] tiles
            holding src.T (partition = old col, free = old row)."""
            out = []
            for it in range(n_colblocks):
                d = persist.tile([128, dst_cols], F32, tag=f"{tag}{it}")
                out.append(d)
            for r, srct in enumerate(src_tiles):
                for it in range(n_colblocks):
                    tp = tp_ps.tile([128, 128], F32, tag="tp")
                    nc.tensor.transpose(tp, srct[:, it * 128:(it + 1) * 128], ident)
                    nc.vector.tensor_copy(out[it][:, r * 128:(r + 1) * 128], tp)
            return out

        wqT = transpose_tiles(wq_raw, IT, HT * 128, "wqT")    # [hin, h]
        wkT = transpose_tiles(wk_raw, IT, HT * 128, "wkT")    # [hin, h]
        queryT = transpose_tiles([query_sb], IT, 128, "qT")   # [hin, qrow]
        keyT = transpose_tiles(key_sb, IT, LK, "kT")          # [hin, krow]

        # ---- projections ----------------------------------------------
        # qpT[ht][h', qrow] = sum_hin Wq[h, hin] * queryT[hin, qrow]  (+bq+bk)
        qpTb = []
        for ht in range(HT):
            pj = pj_ps.tile([128, 128], F32, tag="pj")
            for it in range(IT):
                nc.tensor.matmul(
                    pj, lhsT=wqT[it][:, ht * 128:(ht + 1) * 128], rhs=queryT[it],
                    start=(it == 0), stop=(it == IT - 1),
                )
            t = persist.tile([128, 128], F32, tag=f"qpTb{ht}")
            nc.vector.tensor_scalar_add(t, pj, bsum[ht])
            qpTb.append(t)

        kpT = []
        for ht in range(HT):
            pk = pj_ps.tile([128, LK], F32, tag="pj")
            for it in range(IT):
                nc.tensor.matmul(
                    pk, lhsT=wkT[it][:, ht * 128:(ht + 1) * 128], rhs=keyT[it],
                    start=(it == 0), stop=(it == IT - 1),
                )
            t = persist.tile([128, LK], F32, tag=f"kpT{ht}")
            nc.vector.tensor_copy(t, pk)
            kpT.append(t)

        # ---- main loop: tanh + v-reduction -----------------------------
        # scT[:, ks*128 + q] = scores[q, ks*128 : ks*128+128].T
        # PE output partition base must be 32-aligned, so scores are built
        # transposed: lhsT = tanh tile [h, k_sub] (stationary), rhs = v
        # column (moving), out = [k_sub, 1] at free offset ks*128+q.
        # h-tile accumulation is contiguous per column so the bank-wide
        # has_written clearing of start=True never hits an open group.
        scT = sc_ps.tile([128, KT * 128], F32)

        # sin/tb allocated once per h-tile (not pool-recycled): slot
        # re-acquisition would add extra sync waits beyond what walrus can
        # encode. The ht0/ht1 alternation provides the double buffering.
        sin_t = [sin_pool.tile([128, QB * LK], F32, tag=f"sin{ht}", name=f"sin{ht}")
                 for ht in range(HT)]
        tb_t = [tb_pool.tile([128, QB * LK], F32, tag=f"tb{ht}", name=f"tb{ht}")
                for ht in range(HT)]

        for blk in range(NBLK):
            tbs = []
            for ht in range(HT):
                sin = sin_t[ht]
                for j in range(QB):
                    q = blk * QB + j
                    nc.vector.tensor_scalar_add(
                        sin[:, j * LK:(j + 1) * LK], kpT[ht], qpTb[ht][:, q:q + 1],
                    )
                tb = tb_t[ht]
                nc.scalar.activation(tb, sin, mybir.ActivationFunctionType.Tanh)
                tbs.append(tb)
            for j in range(QB):
                q = blk * QB + j
                for ks in range(KT):
                    col = ks * 128 + q
                    for ht in range(HT):
                        nc.tensor.matmul(
                            scT[:, col:col + 1],
                            lhsT=tbs[ht][:, j * LK + ks * 128:j * LK + (ks + 1) * 128],
                            rhs=v_col[ht],
                            start=(ht == 0), stop=(ht == HT - 1),
                        )

        # ---- recover scores[q, k] via PE transposes --------------------
        scT_sb = persist.tile([128, KT * 128], F32)
        nc.vector.tensor_copy(scT_sb, scT)
        psc = pj_ps.tile([128, LK], F32, tag="pj")
        for ks in range(KT):
            nc.tensor.transpose(
                psc[:, ks * 128:(ks + 1) * 128],
                scT_sb[:, ks * 128:(ks + 1) * 128], ident,
            )

        # ---- masked softmax --------------------------------------------
        sc = persist.tile([128, LK], F32)
        nc.vector.tensor_add(sc, psc, mbias)
        negmax = persist.tile([128, 1], F32)
        nc.vector.reduce_max(negmax, sc, axis=mybir.AxisListType.X, negate=True)
        esb = persist.tile([128, LK], F32)
        rowsum = persist.tile([128, 1], F32)
        nc.scalar.activation(
            esb, sc, mybir.ActivationFunctionType.Exp, bias=negmax,
            accum_out=rowsum,
        )
        rinv = persist.tile([128, 1], F32)
        nc.vector.reciprocal(rinv, rowsum)
        aw = persist.tile([128, LK], F32)
        nc.vector.tensor_scalar_mul(aw, esb, rinv)
        nc.sync.dma_start(out=attn_w[:, :], in_=aw)

        # ---- attn_out = attn_w @ value ---------------------------------
        awT = transpose_tiles([aw], KT, 128, "awT")  # [k, qrow]
        po = out_ps.tile([128, H], F32)
        for kt in range(KT):
            nc.tensor.matmul(
                po, lhsT=awT[kt], rhs=value_sb[kt],
                start=(kt == 0), stop=(kt == KT - 1),
            )
        osb = persist.tile([128, H], F32)
        nc.vector.tensor_copy(osb, po)
        nc.sync.dma_start(out=attn_out[:, :], in_=osb)

    nc.compile()
    return nc


def get_nc():
    if "nc" not in _CACHE:
        _CACHE["nc"] = _build_nc()
    return _CACHE["nc"]


def make_in_maps(query, key, value, mask, Wq, bq, Wk, bk, v, bv=None):
    query = np.ascontiguousarray(np.asarray(query, dtype=np.float32))
    key = np.ascontiguousarray(np.asarray(key, dtype=np.float32))
    value = np.ascontiguousarray(np.asarray(value, dtype=np.float32))
    mask = np.ascontiguousarray(np.asarray(mask, dtype=np.int32))
    Wq = np.ascontiguousarray(np.asarray(Wq, dtype=np.float32))
    bq = np.ascontiguousarray(np.asarray(bq, dtype=np.float32))
    Wk = np.ascontiguousarray(np.asarray(Wk, dtype=np.float32))
    bk = np.ascontiguousarray(np.asarray(bk, dtype=np.float32))
    v = np.ascontiguousarray(np.asarray(v, dtype=np.float32))

    in_maps = []
    for c in range(NCORES):
        b = c // 2
        r0 = (c % 2) * QROWS
        in_maps.append({
            "query": np.ascontiguousarray(query[b, r0:r0 + QROWS, :]),
            "key": key[b],
            "value": value[b],
            "mask": mask[b],
            "Wq": Wq, "bq": bq, "Wk": Wk, "bk": bk, "v": v,
        })
    return in_maps


def assemble(results):
    attn_out = np.empty((B, LQ, H), dtype=np.float32)
    attn_w = np.empty((B, LQ, LK), dtype=np.float32)
    for c in range(NCORES):
        b = c // 2
        r0 = (c % 2) * QROWS
        attn_out[b, r0:r0 + QROWS, :] = results[c]["attn_out"]
        attn_w[b, r0:r0 + QROWS, :] = results[c]["attn_w"]
    return attn_out, attn_w


def kernel(query, key, value, mask, Wq, bq, Wk, bk, v, bv=None):
    from concourse.bass_utils import run_bass_kernel_spmd

    nc = get_nc()
    in_maps = make_in_maps(query, key, value, mask, Wq, bq, Wk, bk, v, bv)
    res = run_bass_kernel_spmd(nc, in_maps, core_ids=list(range(NCORES)))
    return assemble(res.results)


# revision 57
# speedup vs baseline: 1.0068x; 1.0068x over previous
"""Additive (Bahdanau) attention on 8 TRN2 NeuronCores.

reference:
    q = query @ Wq.T + bq                  [B, Lq, H]
    k = key @ Wk.T + bk                    [B, Lk, H]
    scores[b,q,k] = sum_h v[h] * tanh(qp[b,q,h] + kp[b,k,h]) (+ bv)
    scores = where(mask==0, -inf, scores)
    attn_w = softmax(scores, axis=-1)      [B, Lq, Lk]
    attn_out = attn_w @ value              [B, Lq, H]
    returns (attn_out, attn_w)

Sharding: B*Lq = 1024 query rows split 8 ways -> 128 rows/core, each core
gets its batch's key/value/mask. Zero cross-core communication.

Per-core dataflow (h on partitions for the tanh pipeline):
  - PE transposes query/key/Wq/Wk tiles; PE computes qpT[h,q], kpT[h,k].
  - main loop over (h-tile, q-block): DVE tensor_scalar_add broadcasts
    qpT[:,q] over kpT -> big S_in tile; ScalarE tanh (the hard floor:
    Lq*Lk*H/8 = 16.8M elements through the only transcendental engine);
    PE reduces over h with v as a [128,1] stationary operand into PSUM
    score rows.
  - masked softmax on the [128 q, 512 k] PSUM tile (exp shares the tanh
    ACT table set; accum_out yields row sums in the same instruction).
  - PE transposes attn_w, matmuls against value, DMA out.

Note bv is mathematically irrelevant: softmax is shift invariant and
scores are not returned.
"""

from contextlib import ExitStack

import numpy as np

import concourse.bass as bass
import concourse.tile as tile
from concourse import bacc, mybir
from concourse.masks import make_identity

B, LQ, LK, H = 4, 256, 512, 256
NCORES = 8
QROWS = B * LQ // NCORES  # 128 query rows per core
QB = 16                   # query rows per activation block
NEG_BIG = -1e30

F32 = mybir.dt.float32
BF16 = mybir.dt.bfloat16
I32 = mybir.dt.int32
# dtype of the tanh pipeline (kpT/qpTb/sin/tb/v): bf16 gives DVE 4x mode
# on the broadcast adds and halves SBUF; scores accumulate in f32 PSUM.
TDT = BF16

_CACHE: dict = {}


def _build_nc():
    nc = bacc.Bacc()

    # host-side layout prep (make_in_maps): transposed + concatenated into
    # one wide array per DMA so the prologue is a handful of transfers.
    #   wt:   (WqT0 | WqT1 | WkT0 | WkT1)          [128, 4*H]
    #   qt:   (queryT0 | queryT1)                  [128, 2*QROWS]
    #   kt:   (keyT0 | keyT1)                      [128, 2*LK]
    #   vt:   (value[0:128] | ... | value[384:512])[128, 4*H]
    #   cols: (bq0|bq1|bk0|bk1|v0|v1)              [128, 6]
    wt = nc.declare_dram_parameter("wt", [128, 4 * H], TDT, isOutput=False)
    qt = nc.declare_dram_parameter("qt", [128, 2 * QROWS], TDT, isOutput=False)
    kt = nc.declare_dram_parameter("kt", [128, 2 * LK], TDT, isOutput=False)
    vt = nc.declare_dram_parameter("vt", [128, 4 * H], F32, isOutput=False)
    cols = nc.declare_dram_parameter("cols", [128, 6], F32, isOutput=False)
    mask = nc.declare_dram_parameter("mask", [LK], I32, isOutput=False)

    attn_out = nc.declare_dram_parameter("attn_out", [QROWS, H], F32, isOutput=True)
    attn_w = nc.declare_dram_parameter("attn_w", [QROWS, LK], F32, isOutput=True)

    HT = H // 128  # h tiles (2)
    IT = H // 128  # hin tiles (2)
    KT = LK // 128  # key row tiles (4)
    NBLK = QROWS // QB

    with tile.TileContext(nc) as tc, ExitStack() as ctx:
        persist = ctx.enter_context(tc.tile_pool(name="persist", bufs=1))
        sin_pool = ctx.enter_context(tc.tile_pool(name="sin", bufs=2))
        tb_pool = ctx.enter_context(tc.tile_pool(name="tb", bufs=2))
        tp_ps = ctx.enter_context(tc.tile_pool(name="tp_ps", bufs=2, space="PSUM"))
        pj_ps = ctx.enter_context(tc.tile_pool(name="pj_ps", bufs=2, space="PSUM"))
        sc_ps = ctx.enter_context(tc.tile_pool(name="sc_ps", bufs=1, space="PSUM"))
        out_ps = ctx.enter_context(tc.tile_pool(name="out_ps", bufs=2, space="PSUM"))

        # ---- constants / identity / ACT table warmup -------------------
        ident = persist.tile([128, 128], F32)
        make_identity(nc, ident)
        warm = persist.tile([128, 1], F32)
        nc.vector.memset(warm, 0.0)
        # touch the exp_and_others table set early (covers tanh + exp)
        nc.scalar.activation(warm, warm, mybir.ActivationFunctionType.Tanh)

        # ---- packed loads (one DMA each; critical path first) ----------
        kt_sb = persist.tile([128, 2 * LK], TDT)
        nc.sync.dma_start(out=kt_sb, in_=kt[:, :])
        wt_sb = persist.tile([128, 4 * H], TDT)
        nc.sync.dma_start(out=wt_sb, in_=wt[:, :])
        cols_sb = persist.tile([128, 6], F32)
        nc.sync.dma_start(out=cols_sb, in_=cols[:, :])
        qt_sb = persist.tile([128, 2 * QROWS], TDT)
        nc.sync.dma_start(out=qt_sb, in_=qt[:, :])

        wqT = [wt_sb[:, it * H:(it + 1) * H] for it in range(IT)]
        wkT = [wt_sb[:, (2 + it) * H:(3 + it) * H] for it in range(IT)]
        queryT_sb = [qt_sb[:, it * QROWS:(it + 1) * QROWS] for it in range(IT)]
        keyT_sb = [kt_sb[:, it * LK:(it + 1) * LK] for it in range(IT)]
        bq_col = [cols_sb[:, ht:ht + 1] for ht in range(HT)]
        bk_col = [cols_sb[:, 2 + ht:3 + ht] for ht in range(HT)]

        # v columns: bf16 to match the tanh-tile lhsT dtype
        v_col = []
        for ht in range(HT):
            t = persist.tile([128, 1], TDT, tag=f"v{ht}")
            nc.vector.tensor_copy(t, cols_sb[:, 4 + ht:5 + ht])
            v_col.append(t)
        bsum = []
        for ht in range(HT):
            t = persist.tile([128, 1], F32, tag=f"bsum{ht}")
            nc.vector.tensor_add(t, bq_col[ht], bk_col[ht])
            bsum.append(t)

        # ---- projections ----------------------------------------------
        # qpT[ht][h', qrow] = sum_hin Wq[h, hin] * queryT[hin, qrow]  (+bq+bk)
        # kpT first: it is the long pole into the first tanh block.
        # ScalarE does the PSUM->SBUF copies (ACT is idle in the prologue,
        # DVE is busy with the first adds).
        kpT = []
        for ht in range(HT):
            pk = pj_ps.tile([128, LK], F32, tag="pj")
            for it in range(IT):
                nc.tensor.matmul(
                    pk, lhsT=wkT[it][:, ht * 128:(ht + 1) * 128], rhs=keyT_sb[it],
                    start=(it == 0), stop=(it == IT - 1),
                )
            t = persist.tile([128, LK], TDT, tag=f"kpT{ht}")
            nc.vector.tensor_copy(t, pk)
            kpT.append(t)

        qpTb = []
        for ht in range(HT):
            pj = pj_ps.tile([128, 128], F32, tag="pj")
            for it in range(IT):
                nc.tensor.matmul(
                    pj, lhsT=wqT[it][:, ht * 128:(ht + 1) * 128], rhs=queryT_sb[it],
                    start=(it == 0), stop=(it == IT - 1),
                )
            t = persist.tile([128, 128], F32, tag=f"qpTb{ht}")
            nc.vector.tensor_scalar_add(t, pj, bsum[ht])
            qpTb.append(t)

        # ---- main loop: tanh + v-reduction -----------------------------
        # scores are built transposed (PE output partition base must be
        # 32-aligned): scT_h[half][:, ks*64 + q%64] holds
        # scores[q, ks*128:(ks+1)*128].T for q in that 64-row half.
        # lhsT = tanh tile [h, k_sub] (stationary), rhs = v column
        # (moving), out = [k_sub, 1]. h-tile accumulation is contiguous per
        # column so the bank-wide has_written clearing of start=True never
        # hits an open group. Two half tiles live in different PSUM banks
        # so half-0 postprocessing overlaps the second half of the tanh
        # stream without PE-write/DVE-read bank collisions.
        QH = QROWS // 2
        scT_h = [sc_ps.tile([128, KT * QH], F32, name=f"scT{h}", tag=f"scT{h}")
                 for h in range(2)]

        state = {}

        def emit_half_post(half):
            """softmax + attn_w/attn_out for rows [half*QH, (half+1)*QH)."""
            mcol, value_sb = state["mcol"], state["value"]
            r0 = half * QH
            # PSUM -> SBUF eviction fused with the mask add: in the
            # transposed layout the mask bias is per-partition (k on
            # partitions), one tensor_scalar per k-subtile
            scs = persist.tile([128, KT * QH], F32, tag=f"scs{half}",
                               name=f"scs{half}")
            for ks in range(KT):
                nc.vector.tensor_scalar_add(
                    scs[:, ks * QH:(ks + 1) * QH],
                    scT_h[half][:, ks * QH:(ks + 1) * QH],
                    mcol[:, ks:ks + 1],
                )
            psc = pj_ps.tile([QH, LK], F32, tag="pj", name=f"psc{half}")
            for ks in range(KT):
                nc.tensor.transpose(
                    psc[:, ks * 128:(ks + 1) * 128],
                    scs[:, ks * QH:(ks + 1) * QH], ident,
                )
            negmax = persist.tile([QH, 1], F32, tag=f"ngm{half}", name=f"ngm{half}")
            nc.vector.reduce_max(negmax, psc, axis=mybir.AxisListType.X,
                                 negate=True)
            esb = persist.tile([QH, LK], F32, tag=f"esb{half}", name=f"esb{half}")
            rowsum = persist.tile([QH, 1], F32, tag=f"rs{half}", name=f"rs{half}")
            nc.scalar.activation(
                esb, psc, mybir.ActivationFunctionType.Exp, bias=negmax,
                accum_out=rowsum,
            )
            rinv = persist.tile([QH, 1], F32, tag=f"ri{half}", name=f"ri{half}")
            nc.vector.reciprocal(rinv, rowsum)
            aw = persist.tile([QH, LK], F32, tag=f"aw{half}", name=f"aw{half}")
            nc.vector.tensor_scalar_mul(aw, esb, rinv)
            # gpsimd queue: don't serialize behind the attn_out DMA on sync
            nc.gpsimd.dma_start(out=attn_w[r0:r0 + QH, :], in_=aw)

            awT = []
            for kt_i in range(KT):
                d = persist.tile([128, QH], F32, tag=f"awT{half}_{kt_i}",
                                 name=f"awT{half}_{kt_i}")
                tp = tp_ps.tile([128, QH], F32, tag="tp", name="tp")
                nc.tensor.transpose(tp, aw[:, kt_i * 128:(kt_i + 1) * 128],
                                    ident[:QH, :QH])
                nc.vector.tensor_copy(d, tp)
                awT.append(d)
            # h-halves: the first half's copy+DMA overlaps the second's MMs
            for hh in range(2):
                po = out_ps.tile([QH, H // 2], F32, tag="po", name=f"po{half}{hh}")
                for kt_i in range(KT):
                    nc.tensor.matmul(
                        po, lhsT=awT[kt_i],
                        rhs=value_sb[kt_i][:, hh * (H // 2):(hh + 1) * (H // 2)],
                        start=(kt_i == 0), stop=(kt_i == KT - 1),
                    )
                osb = persist.tile([QH, H // 2], F32, tag=f"osb{half}{hh}",
                                   name=f"osb{half}{hh}")
                nc.scalar.copy(osb, po)
                nc.sync.dma_start(
                    out=attn_out[r0:r0 + QH, hh * (H // 2):(hh + 1) * (H // 2)],
                    in_=osb)

        # ramp-up block sizes: tiny first blocks so the tanh stream starts
        # as soon as kpT/qpTb land; steady-state blocks amortize overheads.
        BLOCKS = [2, 2, 4, 8, 16, 16, 16, 16, 16, 16, 16]
        assert sum(BLOCKS) == QROWS
        q0 = 0
        for blk, qb in enumerate(BLOCKS):
            if blk == 1:
                # emitted here so the scheduler runs these loads during the
                # main loop (off both the prologue and tail critical paths)
                vt_sb = persist.tile([128, 4 * H], F32)
                nc.sync.dma_start(out=vt_sb, in_=vt[:, :])
                state["value"] = [vt_sb[:, k * H:(k + 1) * H] for k in range(KT)]
                # mask as a single [1, LK] additive-bias row, folded into
                # the scores PSUM by rank-1 accumulate matmuls
                # mask as [128, KT] columns (k on partitions, one column
                # per k-subtile) -> additive bias in the scT layout
                mask_i = persist.tile([128, KT], I32)
                nc.sync.dma_start(
                    out=mask_i,
                    in_=bass.AP(tensor=mask, offset=0, ap=[[1, 128], [128, KT]]))
                mcol = persist.tile([128, KT], F32)
                # mask==1 -> 0.0 ; mask==0 -> NEG_BIG
                nc.vector.tensor_scalar(
                    out=mcol, in0=mask_i, scalar1=-NEG_BIG, scalar2=NEG_BIG,
                    op0=mybir.AluOpType.mult, op1=mybir.AluOpType.add,
                )
                state["mcol"] = mcol
            tbs = []
            for ht in range(HT):
                sin = sin_pool.tile([128, qb * LK], TDT, tag=f"sin{ht}",
                                    name=f"sin{ht}")
                for j in range(qb):
                    q = q0 + j
                    nc.vector.tensor_scalar_add(
                        sin[:, j * LK:(j + 1) * LK], kpT[ht], qpTb[ht][:, q:q + 1],
                    )
                tb = tb_pool.tile([128, qb * LK], TDT, tag=f"tb{ht}",
                                  name=f"tb{ht}")
                nc.scalar.activation(tb, sin, mybir.ActivationFunctionType.Tanh)
                tbs.append(tb)
            for j in range(qb):
                q = q0 + j
                half, ql = q // QH, q % QH
                for ks in range(KT):
                    col = ks * QH + ql
                    for ht in range(HT):
                        nc.tensor.matmul(
                            scT_h[half][:, col:col + 1],
                            lhsT=tbs[ht][:, j * LK + ks * 128:j * LK + (ks + 1) * 128],
                            rhs=v_col[ht],
                            start=(ht == 0), stop=(ht == HT - 1),
                        )
            q0 += qb
            if q0 == QH:
                emit_half_post(0)
        emit_half_post(1)

    nc.compile()
    return nc


def get_nc():
    if "nc" not in _CACHE:
        _CACHE["nc"] = _build_nc()
    return _CACHE["nc"]


def make_in_maps(query, key, value, mask, Wq, bq, Wk, bk, v, bv=None):
    query = np.ascontiguousarray(np.asarray(query, dtype=np.float32))
    key = np.ascontiguousarray(np.asarray(key, dtype=np.float32))
    value = np.ascontiguousarray(np.asarray(value, dtype=np.float32))
    mask = np.ascontiguousarray(np.asarray(mask, dtype=np.int32))
    Wq = np.ascontiguousarray(np.asarray(Wq, dtype=np.float32))
    bq = np.ascontiguousarray(np.asarray(bq, dtype=np.float32))
    Wk = np.ascontiguousarray(np.asarray(Wk, dtype=np.float32))
    bk = np.ascontiguousarray(np.asarray(bk, dtype=np.float32))
    v = np.ascontiguousarray(np.asarray(v, dtype=np.float32))

    from concourse import mybir as _mybir
    bf16 = _mybir.dt.np(TDT)

    WqT = Wq.T
    WkT = Wk.T
    # wt = (WqT0 | WqT1 | WkT0 | WkT1), each [128, H]
    wt = np.ascontiguousarray(np.concatenate(
        [WqT[:128], WqT[128:], WkT[:128], WkT[128:]], axis=1)).astype(bf16)
    # cols = (bq0|bq1|bk0|bk1|v0|v1)
    cols = np.ascontiguousarray(np.stack(
        [bq[:128], bq[128:], bk[:128], bk[128:], v[:128], v[128:]], axis=1))

    kt_b = {}
    vt_b = {}
    for b in range(B):
        keyT = key[b].T  # [H, LK]
        kt_b[b] = np.ascontiguousarray(np.concatenate(
            [keyT[:128], keyT[128:]], axis=1)).astype(bf16)
        vt_b[b] = np.ascontiguousarray(np.concatenate(
            [value[b, k * 128:(k + 1) * 128, :] for k in range(4)], axis=1))

    in_maps = []
    for c in range(NCORES):
        b = c // 2
        r0 = (c % 2) * QROWS
        qT = query[b, r0:r0 + QROWS, :].T  # [H, QROWS]
        qt = np.ascontiguousarray(
            np.concatenate([qT[:128], qT[128:]], axis=1)).astype(bf16)
        in_maps.append({
            "qt": qt,
            "kt": kt_b[b],
            "vt": vt_b[b],
            "mask": mask[b],
            "wt": wt,
            "cols": cols,
        })
    return in_maps


def assemble(results):
    attn_out = np.empty((B, LQ, H), dtype=np.float32)
    attn_w = np.empty((B, LQ, LK), dtype=np.float32)
    for c in range(NCORES):
        b = c // 2
        r0 = (c % 2) * QROWS
        attn_out[b, r0:r0 + QROWS, :] = results[c]["attn_out"]
        attn_w[b, r0:r0 + QROWS, :] = results[c]["attn_w"]
    return attn_out, attn_w


def kernel(query, key, value, mask, Wq, bq, Wk, bk, v, bv=None):
    from concourse.bass_utils import run_bass_kernel_spmd

    nc = get_nc()
    in_maps = make_in_maps(query, key, value, mask, Wq, bq, Wk, bk, v, bv)
    res = run_bass_kernel_spmd(nc, in_maps, core_ids=list(range(NCORES)))
    return assemble(res.results)


# revision 59
# speedup vs baseline: 344.9244x; 342.5844x over previous
"""Additive (Bahdanau) attention on 8 TRN2 NeuronCores.

reference:
    q = query @ Wq.T + bq                  [B, Lq, H]
    k = key @ Wk.T + bk                    [B, Lk, H]
    scores[b,q,k] = sum_h v[h] * tanh(qp[b,q,h] + kp[b,k,h]) (+ bv)
    scores = where(mask==0, -inf, scores)
    attn_w = softmax(scores, axis=-1)      [B, Lq, Lk]
    attn_out = attn_w @ value              [B, Lq, H]
    returns (attn_out, attn_w)

Sharding: B*Lq = 1024 query rows split 8 ways -> 128 rows/core, each core
gets its batch's key/value/mask. Zero cross-core communication.

Per-core dataflow (h on partitions for the tanh pipeline):
  - PE transposes query/key/Wq/Wk tiles; PE computes qpT[h,q], kpT[h,k].
  - main loop over (h-tile, q-block): DVE tensor_scalar_add broadcasts
    qpT[:,q] over kpT -> big S_in tile; ScalarE tanh (the hard floor:
    Lq*Lk*H/8 = 16.8M elements through the only transcendental engine);
    PE reduces over h with v as a [128,1] stationary operand into PSUM
    score rows.
  - masked softmax on the [128 q, 512 k] PSUM tile (exp shares the tanh
    ACT table set; accum_out yields row sums in the same instruction).
  - PE transposes attn_w, matmuls against value, DMA out.

Note bv is mathematically irrelevant: softmax is shift invariant and
scores are not returned.
"""

from contextlib import ExitStack

import numpy as np

import concourse.bass as bass
import concourse.tile as tile
from concourse import bacc, mybir
from concourse.masks import make_identity

B, LQ, LK, H = 4, 256, 512, 256
NCORES = 8
QROWS = B * LQ // NCORES  # 128 query rows per core
QB = 16                   # query rows per activation block
NEG_BIG = -1e30

F32 = mybir.dt.float32
BF16 = mybir.dt.bfloat16
I32 = mybir.dt.int32
# dtype of the tanh pipeline (kpT/qpTb/sin/tb/v): bf16 gives DVE 4x mode
# on the broadcast adds and halves SBUF; scores accumulate in f32 PSUM.
TDT = BF16

_CACHE: dict = {}


def _build_nc():
    nc = bacc.Bacc()

    # host-side layout prep (make_in_maps): transposed + concatenated into
    # one wide array per DMA so the prologue is a handful of transfers.
    #   wt:   (WqT0 | WqT1 | WkT0 | WkT1)          [128, 4*H]
    #   qt:   (queryT0 | queryT1)                  [128, 2*QROWS]
    #   kt:   (keyT0 | keyT1)                      [128, 2*LK]
    #   vt:   (value[0:128] | ... | value[384:512])[128, 4*H]
    #   cols: (bq0|bq1|bk0|bk1|v0|v1)              [128, 6]
    wt = nc.declare_dram_parameter("wt", [128, 4 * H], TDT, isOutput=False)
    qt = nc.declare_dram_parameter("qt", [128, 2 * QROWS], TDT, isOutput=False)
    kt = nc.declare_dram_parameter("kt", [128, 2 * LK], TDT, isOutput=False)
    vt = nc.declare_dram_parameter("vt", [128, 4 * H], F32, isOutput=False)
    cols = nc.declare_dram_parameter("cols", [128, 6], F32, isOutput=False)
    mask = nc.declare_dram_parameter("mask", [LK], I32, isOutput=False)

    attn_out = nc.declare_dram_parameter("attn_out", [QROWS, H], F32, isOutput=True)
    attn_w = nc.declare_dram_parameter("attn_w", [QROWS, LK], F32, isOutput=True)

    HT = H // 128  # h tiles (2)
    IT = H // 128  # hin tiles (2)
    KT = LK // 128  # key row tiles (4)
    NBLK = QROWS // QB

    with tile.TileContext(nc) as tc, ExitStack() as ctx:
        persist = ctx.enter_context(tc.tile_pool(name="persist", bufs=1))
        sin_pool = ctx.enter_context(tc.tile_pool(name="sin", bufs=2))
        tb_pool = ctx.enter_context(tc.tile_pool(name="tb", bufs=2))
        tp_ps = ctx.enter_context(tc.tile_pool(name="tp_ps", bufs=2, space="PSUM"))
        pj_ps = ctx.enter_context(tc.tile_pool(name="pj_ps", bufs=2, space="PSUM"))
        sc_ps = ctx.enter_context(tc.tile_pool(name="sc_ps", bufs=1, space="PSUM"))
        out_ps = ctx.enter_context(tc.tile_pool(name="out_ps", bufs=2, space="PSUM"))

        # ---- constants / identity / ACT table warmup -------------------
        ident = persist.tile([128, 128], F32)
        make_identity(nc, ident)
        warm = persist.tile([128, 1], F32)
        nc.vector.memset(warm, 0.0)
        # touch the exp_and_others table set early (covers tanh + exp)
        nc.scalar.activation(warm, warm, mybir.ActivationFunctionType.Tanh)

        # ---- packed loads (one DMA each; critical path first) ----------
        kt_sb = persist.tile([128, 2 * LK], TDT)
        nc.sync.dma_start(out=kt_sb, in_=kt[:, :])
        wt_sb = persist.tile([128, 4 * H], TDT)
        nc.gpsimd.dma_start(out=wt_sb, in_=wt[:, :])  # parallel DMA queue
        cols_sb = persist.tile([128, 6], F32)
        nc.sync.dma_start(out=cols_sb, in_=cols[:, :])
        qt_sb = persist.tile([128, 2 * QROWS], TDT)
        nc.sync.dma_start(out=qt_sb, in_=qt[:, :])

        wqT = [wt_sb[:, it * H:(it + 1) * H] for it in range(IT)]
        wkT = [wt_sb[:, (2 + it) * H:(3 + it) * H] for it in range(IT)]
        queryT_sb = [qt_sb[:, it * QROWS:(it + 1) * QROWS] for it in range(IT)]
        keyT_sb = [kt_sb[:, it * LK:(it + 1) * LK] for it in range(IT)]
        bq_col = [cols_sb[:, ht:ht + 1] for ht in range(HT)]
        bk_col = [cols_sb[:, 2 + ht:3 + ht] for ht in range(HT)]

        # v columns: bf16 to match the tanh-tile lhsT dtype
        v_col = []
        for ht in range(HT):
            t = persist.tile([128, 1], TDT, tag=f"v{ht}")
            nc.vector.tensor_copy(t, cols_sb[:, 4 + ht:5 + ht])
            v_col.append(t)
        bsum = []
        for ht in range(HT):
            t = persist.tile([128, 1], F32, tag=f"bsum{ht}")
            nc.vector.tensor_add(t, bq_col[ht], bk_col[ht])
            bsum.append(t)

        # ---- projections ----------------------------------------------
        # qpT[ht][h', qrow] = sum_hin Wq[h, hin] * queryT[hin, qrow]  (+bq+bk)
        # kpT first: it is the long pole into the first tanh block.
        # ScalarE does the PSUM->SBUF copies (ACT is idle in the prologue,
        # DVE is busy with the first adds).
        kpT = []
        for ht in range(HT):
            pk = pj_ps.tile([128, LK], F32, tag="pj")
            for it in range(IT):
                nc.tensor.matmul(
                    pk, lhsT=wkT[it][:, ht * 128:(ht + 1) * 128], rhs=keyT_sb[it],
                    start=(it == 0), stop=(it == IT - 1),
                )
            t = persist.tile([128, LK], TDT, tag=f"kpT{ht}")
            nc.vector.tensor_copy(t, pk)
            kpT.append(t)

        qpTb = []
        for ht in range(HT):
            pj = pj_ps.tile([128, 128], F32, tag="pj")
            for it in range(IT):
                nc.tensor.matmul(
                    pj, lhsT=wqT[it][:, ht * 128:(ht + 1) * 128], rhs=queryT_sb[it],
                    start=(it == 0), stop=(it == IT - 1),
                )
            t = persist.tile([128, 128], F32, tag=f"qpTb{ht}")
            nc.vector.tensor_scalar_add(t, pj, bsum[ht])
            qpTb.append(t)

        # ---- main loop: tanh + v-reduction -----------------------------
        # scores are built transposed (PE output partition base must be
        # 32-aligned): scT_h[half][:, ks*64 + q%64] holds
        # scores[q, ks*128:(ks+1)*128].T for q in that 64-row half.
        # lhsT = tanh tile [h, k_sub] (stationary), rhs = v column
        # (moving), out = [k_sub, 1]. h-tile accumulation is contiguous per
        # column so the bank-wide has_written clearing of start=True never
        # hits an open group. Two half tiles live in different PSUM banks
        # so half-0 postprocessing overlaps the second half of the tanh
        # stream without PE-write/DVE-read bank collisions.
        QH = QROWS // 2
        scT_h = [sc_ps.tile([128, KT * QH], F32, name=f"scT{h}", tag=f"scT{h}")
                 for h in range(2)]

        state = {}

        def emit_half_post(half):
            """softmax + attn_w/attn_out for rows [half*QH, (half+1)*QH)."""
            mcol, value_sb = state["mcol"], state["value"]
            r0 = half * QH
            # PSUM -> SBUF eviction fused with the mask add: in the
            # transposed layout the mask bias is per-partition (k on
            # partitions), one tensor_scalar per k-subtile
            scs = persist.tile([128, KT * QH], F32, tag=f"scs{half}",
                               name=f"scs{half}")
            for ks in range(KT):
                nc.vector.tensor_scalar_add(
                    scs[:, ks * QH:(ks + 1) * QH],
                    scT_h[half][:, ks * QH:(ks + 1) * QH],
                    mcol[:, ks:ks + 1],
                )
            psc = pj_ps.tile([QH, LK], F32, tag="pj", name=f"psc{half}")
            for ks in range(KT):
                nc.tensor.transpose(
                    psc[:, ks * 128:(ks + 1) * 128],
                    scs[:, ks * QH:(ks + 1) * QH], ident,
                )
            negmax = persist.tile([QH, 1], F32, tag=f"ngm{half}", name=f"ngm{half}")
            nc.vector.reduce_max(negmax, psc, axis=mybir.AxisListType.X,
                                 negate=True)
            esb = persist.tile([QH, LK], F32, tag=f"esb{half}", name=f"esb{half}")
            rowsum = persist.tile([QH, 1], F32, tag=f"rs{half}", name=f"rs{half}")
            nc.scalar.activation(
                esb, psc, mybir.ActivationFunctionType.Exp, bias=negmax,
                accum_out=rowsum,
            )
            rinv = persist.tile([QH, 1], F32, tag=f"ri{half}", name=f"ri{half}")
            nc.vector.reciprocal(rinv, rowsum)
            aw = persist.tile([QH, LK], F32, tag=f"aw{half}", name=f"aw{half}")
            nc.vector.tensor_scalar_mul(aw, esb, rinv)
            # gpsimd queue: don't serialize behind the attn_out DMA on sync
            nc.gpsimd.dma_start(out=attn_w[r0:r0 + QH, :], in_=aw)

            awT = []
            for kt_i in range(KT):
                d = persist.tile([128, QH], F32, tag=f"awT{half}_{kt_i}",
                                 name=f"awT{half}_{kt_i}")
                tp = tp_ps.tile([128, QH], F32, tag="tp", name="tp")
                nc.tensor.transpose(tp, aw[:, kt_i * 128:(kt_i + 1) * 128],
                                    ident[:QH, :QH])
                nc.vector.tensor_copy(d, tp)
                awT.append(d)
            # h-halves: the first half's copy+DMA overlaps the second's MMs
            for hh in range(2):
                po = out_ps.tile([QH, H // 2], F32, tag="po", name=f"po{half}{hh}")
                for kt_i in range(KT):
                    nc.tensor.matmul(
                        po, lhsT=awT[kt_i],
                        rhs=value_sb[kt_i][:, hh * (H // 2):(hh + 1) * (H // 2)],
                        start=(kt_i == 0), stop=(kt_i == KT - 1),
                    )
                osb = persist.tile([QH, H // 2], F32, tag=f"osb{half}{hh}",
                                   name=f"osb{half}{hh}")
                nc.scalar.copy(osb, po)
                nc.sync.dma_start(
                    out=attn_out[r0:r0 + QH, hh * (H // 2):(hh + 1) * (H // 2)],
                    in_=osb)

        # ramp-up block sizes: tiny first blocks so the tanh stream starts
        # as soon as kpT/qpTb land; steady-state blocks amortize overheads.
        BLOCKS = [1, 1, 2, 4, 8, 16, 16, 16, 16, 16, 16, 16]
        assert sum(BLOCKS) == QROWS
        q0 = 0
        for blk, qb in enumerate(BLOCKS):
            if blk == 1:
                # emitted here so the scheduler runs these loads during the
                # main loop (off both the prologue and tail critical paths)
                vt_sb = persist.tile([128, 4 * H], F32)
                nc.sync.dma_start(out=vt_sb, in_=vt[:, :])
                state["value"] = [vt_sb[:, k * H:(k + 1) * H] for k in range(KT)]
                # mask as a single [1, LK] additive-bias row, folded into
                # the scores PSUM by rank-1 accumulate matmuls
                # mask as [128, KT] columns (k on partitions, one column
                # per k-subtile) -> additive bias in the scT layout
                mask_i = persist.tile([128, KT], I32)
                nc.sync.dma_start(
                    out=mask_i,
                    in_=bass.AP(tensor=mask, offset=0, ap=[[1, 128], [128, KT]]))
                mcol = persist.tile([128, KT], F32)
                # mask==1 -> 0.0 ; mask==0 -> NEG_BIG
                nc.vector.tensor_scalar(
                    out=mcol, in0=mask_i, scalar1=-NEG_BIG, scalar2=NEG_BIG,
                    op0=mybir.AluOpType.mult, op1=mybir.AluOpType.add,
                )
                state["mcol"] = mcol
            tbs = []
            for ht in range(HT):
                sin = sin_pool.tile([128, qb * LK], TDT, tag=f"sin{ht}",
                                    name=f"sin{ht}")
                for j in range(qb):
                    q = q0 + j
                    nc.vector.tensor_scalar_add(
                        sin[:, j * LK:(j + 1) * LK], kpT[ht], qpTb[ht][:, q:q + 1],
                    )
                tb = tb_pool.tile([128, qb * LK], TDT, tag=f"tb{ht}",
                                  name=f"tb{ht}")
                nc.scalar.activation(tb, sin, mybir.ActivationFunctionType.Tanh)
                tbs.append(tb)
            for j in range(qb):
                q = q0 + j
                half, ql = q // QH, q % QH
                for ks in range(KT):
                    col = ks * QH + ql
                    for ht in range(HT):
                        nc.tensor.matmul(
                            scT_h[half][:, col:col + 1],
                            lhsT=tbs[ht][:, j * LK + ks * 128:j * LK + (ks + 1) * 128],
                            rhs=v_col[ht],
                            start=(ht == 0), stop=(ht == HT - 1),
                        )
            q0 += qb
            if q0 == QH:
                emit_half_post(0)
        emit_half_post(1)

    nc.compile()
    return nc


def get_nc():
    if "nc" not in _CACHE:
        _CACHE["nc"] = _build_nc()
    return _CACHE["nc"]


def make_in_maps(query, key, value, mask, Wq, bq, Wk, bk, v, bv=None):
    query = np.ascontiguousarray(np.asarray(query, dtype=np.float32))
    key = np.ascontiguousarray(np.asarray(key, dtype=np.float32))
    value = np.ascontiguousarray(np.asarray(value, dtype=np.float32))
    mask = np.ascontiguousarray(np.asarray(mask, dtype=np.int32))
    Wq = np.ascontiguousarray(np.asarray(Wq, dtype=np.float32))
    bq = np.ascontiguousarray(np.asarray(bq, dtype=np.float32))
    Wk = np.ascontiguousarray(np.asarray(Wk, dtype=np.float32))
    bk = np.ascontiguousarray(np.asarray(bk, dtype=np.float32))
    v = np.ascontiguousarray(np.asarray(v, dtype=np.float32))

    from concourse import mybir as _mybir
    bf16 = _mybir.dt.np(TDT)

    WqT = Wq.T
    WkT = Wk.T
    # wt = (WqT0 | WqT1 | WkT0 | WkT1), each [128, H]
    wt = np.ascontiguousarray(np.concatenate(
        [WqT[:128], WqT[128:], WkT[:128], WkT[128:]], axis=1)).astype(bf16)
    # cols = (bq0|bq1|bk0|bk1|v0|v1)
    cols = np.ascontiguousarray(np.stack(
        [bq[:128], bq[128:], bk[:128], bk[128:], v[:128], v[128:]], axis=1))

    kt_b = {}
    vt_b = {}
    for b in range(B):
        keyT = key[b].T  # [H, LK]
        kt_b[b] = np.ascontiguousarray(np.concatenate(
            [keyT[:128], keyT[128:]], axis=1)).astype(bf16)
        vt_b[b] = np.ascontiguousarray(np.concatenate(
            [value[b, k * 128:(k + 1) * 128, :] for k in range(4)], axis=1))

    in_maps = []
    for c in range(NCORES):
        b = c // 2
        r0 = (c % 2) * QROWS
        qT = query[b, r0:r0 + QROWS, :].T  # [H, QROWS]
        qt = np.ascontiguousarray(
            np.concatenate([qT[:128], qT[128:]], axis=1)).astype(bf16)
        in_maps.append({
            "qt": qt,
            "kt": kt_b[b],
            "vt": vt_b[b],
            "mask": mask[b],
            "wt": wt,
            "cols": cols,
        })
    return in_maps


def assemble(results):
    attn_out = np.empty((B, LQ, H), dtype=np.float32)
    attn_w = np.empty((B, LQ, LK), dtype=np.float32)
    for c in range(NCORES):
        b = c // 2
        r0 = (c % 2) * QROWS
        attn_out[b, r0:r0 + QROWS, :] = results[c]["attn_out"]
        attn_w[b, r0:r0 + QROWS, :] = results[c]["attn_w"]
    return attn_out, attn_w


def kernel(query, key, value, mask, Wq, bq, Wk, bk, v, bv=None):
    from concourse.bass_utils import run_bass_kernel_spmd

    nc = get_nc()
    in_maps = make_in_maps(query, key, value, mask, Wq, bq, Wk, bk, v, bv)
    res = run_bass_kernel_spmd(nc, in_maps, core_ids=list(range(NCORES)))
    return assemble(res.results)
